# revision 6
# baseline (speedup 1.0000x reference)
"""GCNNet (SimpleConv sum-aggr + global_mean_pool + 2-layer MLP) on 8 trn2 cores.

Math: out[g] = MLP(relu(sums[g] / max(counts[g],1)))
  sums[g,:]  = sum_e w_e * x[src_e,:] * [batch[dst_e]==g]
  counts[g]  = #{i : batch[i]==g}

Sharding (v2, src-parallel): core k owns src rows [6250k, 6250(k+1)).  The
per-core sparse cell matrix A_k[src, g] (coalesced edge weights) is laid out as
49 dense windows [128, 512] quantized to uint8 with one global scale s; the
device casts u8->fp16 during the (SWDGE) DMA and accumulates
acc[96, 512] += x_w^T @ C_w over windows in f32 PSUM.  Node counts for the
core's own graph range come from 0/1-layer matrices carrying 1/s so the
epilogue's reciprocal absorbs the quantization scale.  Partial acc+counts
[97, 512] are summed across the 8 cores with an on-device collective, then
every core runs the tiny-MLP epilogue.
"""

import numpy as np

N_NODES = 50000
N_EDGES = 800000
D_FEAT = 96
D_HID = 10
N_GRAPHS = 512
CORES = 8
RPC = N_NODES // CORES          # 6250 src rows per core
GPC = N_GRAPHS // CORES         # 64 graphs per core
P = 128
NW = (RPC + P - 1) // P         # 49 windows
G = N_GRAPHS

# "AR": AllReduce [97,512], every core runs the full epilogue, host takes core0.
# "RS": ReduceScatter, each core gets its 64-graph slab, host concats.
# "A2A": AllToAll slab exchange + on-core DVE adds (cheapest), host concats.
COLLECTIVE = "A2A"
N_CAST_CHUNKS = 7               # cu cast-DMA pipeline chunks

_nc_cache = {}


def _build_nc(n_layers, collective):
    import concourse.mybir as mybir
    import concourse.tile as tile
    from concourse import bacc

    f32 = mybir.dt.float32
    f16 = mybir.dt.float16
    u8 = mybir.dt.uint8
    L = n_layers

    nc = bacc.Bacc(
        "TRN2",
        target_bir_lowering=False,
        debug=False,
        num_devices=CORES,
    )

    cu_d = nc.dram_tensor("cu", [P, NW * G], u8, kind="ExternalInput")
    xw_d = nc.dram_tensor("xw", [P, NW * D_FEAT], f16, kind="ExternalInput")
    cm_d = nc.dram_tensor("cm", [P, L * G], f16, kind="ExternalInput")
    w1_d = nc.dram_tensor("w1", [D_FEAT, D_HID], f32, kind="ExternalInput")
    b1_d = nc.dram_tensor("b1", [D_HID, 1], f32, kind="ExternalInput")
    w2_d = nc.dram_tensor("w2", [D_HID, 1], f32, kind="ExternalInput")
    b2_d = nc.dram_tensor("b2", [1, 1], f32, kind="ExternalInput")
    GOUT = G if collective == "AR" else GPC
    out_d = nc.dram_tensor("out", [1, GOUT], f32, kind="ExternalOutput")

    # window chunks for the cast-DMA pipeline (front-loaded small chunks)
    sizes = []
    w = 0
    ramp = [2, 3, 5, 7]
    i = 0
    while w < NW:
        n = min(ramp[i] if i < len(ramp) else 10, NW - w)
        sizes.append(n)
        w += n
        i += 1

    with tile.TileContext(nc) as tc:
        with (
            tc.tile_pool(name="const", bufs=1) as cp,
            tc.tile_pool(name="cw", bufs=len(sizes)) as cwp,
            tc.tile_pool(name="psum", bufs=1, space="PSUM") as pp,
            tc.tile_pool(name="dram", bufs=1, space="DRAM") as dram,
        ):
            acc_ps = pp.tile([D_FEAT, G], f32, tag="acc")
            cnt_ps = pp.tile([1, G], f32, tag="cnt")

            # x windows: two DMAs so the first windows arrive early
            xw_t = cp.tile([P, NW * D_FEAT], f16, tag="xw")
            nxh = 8
            nc.sync.dma_start(
                out=xw_t[:, : nxh * D_FEAT], in_=xw_d[:, : nxh * D_FEAT]
            )
            nc.sync.dma_start(
                out=xw_t[:, nxh * D_FEAT :], in_=xw_d[:, nxh * D_FEAT :]
            )

            ones_t = cp.tile([P, 1], f16, tag="ones")
            nc.vector.memset(ones_t[:], 1.0)
            ones10_t = cp.tile([1, D_HID], f32, tag="ones10")
            nc.vector.memset(ones10_t[:], 1.0)

            cm_t = cp.tile([P, L * G], f16, tag="cm")
            nc.sync.dma_start(out=cm_t[:], in_=cm_d[:, :])
            w1_t = cp.tile([D_FEAT, D_HID], f32, tag="w1")
            nc.sync.dma_start(out=w1_t[:], in_=w1_d[:, :])
            b1_t = cp.tile([D_HID, 1], f32, tag="b1")
            nc.sync.dma_start(out=b1_t[:], in_=b1_d[:, :])
            w2_t = cp.tile([D_HID, 1], f32, tag="w2")
            nc.sync.dma_start(out=w2_t[:], in_=w2_d[:, :])
            b2_t = cp.tile([1, 1], f32, tag="b2")
            nc.sync.dma_start(out=b2_t[:], in_=b2_d[:, :])

            # main loop: cast-DMA chunk of C windows, then matmul-accumulate
            w0 = 0
            for ci, ncw in enumerate(sizes):
                cw_t = cwp.tile([P, 10 * G], f16, tag="cwt")
                nc.gpsimd.dma_start(
                    out=cw_t[:, : ncw * G],
                    in_=cu_d[:, w0 * G : (w0 + ncw) * G],
                )
                for lw in range(ncw):
                    w = w0 + lw
                    nc.tensor.matmul(
                        acc_ps[:, :],
                        lhsT=xw_t[:, w * D_FEAT : (w + 1) * D_FEAT],
                        rhs=cw_t[:, lw * G : (lw + 1) * G],
                        start=(w == 0),
                        stop=(w == NW - 1),
                    )
                w0 += ncw

            # node counts (value 1/s baked into cm)
            for l in range(L):
                nc.tensor.matmul(
                    cnt_ps[:, :],
                    lhsT=ones_t[:],
                    rhs=cm_t[:, l * G : (l + 1) * G],
                    start=(l == 0),
                    stop=(l == L - 1),
                )

            # stage partials [97, 512] and run the collective through DRAM
            part_sb = cp.tile([D_FEAT + 1, G], f32, tag="part")
            nc.vector.tensor_copy(out=part_sb[:D_FEAT, :], in_=acc_ps[:, :])
            nc.vector.tensor_copy(
                out=part_sb[D_FEAT : D_FEAT + 1, :], in_=cnt_ps[:, :]
            )

            PF = D_FEAT + 1
            CB = PF * GPC
            if collective == "AR":
                ar_in = dram.tile([PF, G], f32, tag="ar_in")
                ar_out = dram.tile([PF, G], f32, tag="ar_out", addr_space="Shared")
                nc.sync.dma_start(out=ar_in[:], in_=part_sb[:])
                nc.gpsimd.collective_compute(
                    "AllReduce",
                    mybir.AluOpType.add,
                    replica_groups=[list(range(CORES))],
                    ins=[ar_in[:].opt()],
                    outs=[ar_out[:].opt()],
                )
                all_sb = cp.tile([PF, G], f32, tag="all")
                nc.sync.dma_start(out=all_sb[:], in_=ar_out[:])
            elif collective == "RS":
                # slab-major staging: DRAM chunk k holds part_sb[:, 64k:64(k+1)]
                rs_in = dram.tile([CORES, CB], f32, tag="rs_in")
                rs_out = dram.tile([1, CB], f32, tag="rs_out")
                nc.sync.dma_start(
                    out=rs_in[:].rearrange("i (p c) -> p i c", p=PF, c=GPC),
                    in_=part_sb[:].rearrange("p (i c) -> p i c", i=CORES, c=GPC),
                )
                nc.gpsimd.collective_compute(
                    "ReduceScatter",
                    mybir.AluOpType.add,
                    replica_groups=[list(range(CORES))],
                    ins=[rs_in[:].opt()],
                    outs=[rs_out[:].opt()],
                )
                all_sb = cp.tile([PF, GPC], f32, tag="all")
                nc.sync.dma_start(
                    out=all_sb[:],
                    in_=rs_out[:].rearrange("o (p c) -> (o p) c", p=PF, c=GPC),
                )
            else:  # A2A: chunk j of rank i's input lands as chunk i on rank j
                aa_in = dram.tile([CORES, CB], f32, tag="aa_in")
                aa_out = dram.tile([CORES, CB], f32, tag="aa_out")
                nc.sync.dma_start(
                    out=aa_in[:].rearrange("i (p c) -> p i c", p=PF, c=GPC),
                    in_=part_sb[:].rearrange("p (i c) -> p i c", i=CORES, c=GPC),
                )
                nc.gpsimd.collective_compute(
                    "AllToAll",
                    mybir.AluOpType.bypass,
                    replica_groups=[list(range(CORES))],
                    ins=[aa_in[:].opt()],
                    outs=[aa_out[:].opt()],
                )
                all8_sb = cp.tile([PF, CORES * GPC], f32, tag="all8")
                nc.sync.dma_start(
                    out=all8_sb[:].rearrange("p (i c) -> p i c", i=CORES, c=GPC),
                    in_=aa_out[:].rearrange("i (p c) -> p i c", p=PF, c=GPC),
                )
                # tree-sum the 8 received partial slabs -> [97, 64]
                h1 = cp.tile([PF, 4 * GPC], f32, tag="h1")
                nc.vector.tensor_tensor(
                    h1[:], all8_sb[:, : 4 * GPC], all8_sb[:, 4 * GPC :],
                    mybir.AluOpType.add,
                )
                h2 = cp.tile([PF, 2 * GPC], f32, tag="h2")
                nc.vector.tensor_tensor(
                    h2[:], h1[:, : 2 * GPC], h1[:, 2 * GPC :], mybir.AluOpType.add
                )
                all_sb = cp.tile([PF, GPC], f32, tag="all")
                nc.vector.tensor_tensor(
                    all_sb[:], h2[:, :GPC], h2[:, GPC:], mybir.AluOpType.add
                )

            # epilogue: relu commutes with the positive 1/(s*count) scale
            GE = GOUT
            a_sb = cp.tile([D_FEAT, GE], f32, tag="a")
            nc.vector.tensor_scalar_max(a_sb[:], all_sb[:D_FEAT, :], 0.0)
            cmax = cp.tile([1, GE], f32, tag="cmax")
            nc.vector.tensor_scalar_max(
                cmax[:], all_sb[D_FEAT : D_FEAT + 1, :], 1.0
            )
            recip = cp.tile([1, GE], f32, tag="recip")
            nc.vector.reciprocal(recip[:], cmax[:])

            b_ps = pp.tile([D_HID, GE], f32, tag="b")
            nc.tensor.matmul(b_ps[:, :], lhsT=w1_t[:], rhs=a_sb[:], start=True, stop=True)
            rb_ps = pp.tile([D_HID, GE], f32, tag="rb")
            nc.tensor.matmul(
                rb_ps[:, :], lhsT=ones10_t[:], rhs=recip[:], start=True, stop=True
            )
            rb_sb = cp.tile([D_HID, GE], f32, tag="rbs")
            nc.vector.tensor_copy(out=rb_sb[:, :], in_=rb_ps[:, :])

            z_sb = cp.tile([D_HID, GE], f32, tag="z")
            nc.vector.tensor_tensor(
                z_sb[:], b_ps[:, :], rb_sb[:], mybir.AluOpType.mult
            )
            nc.vector.tensor_scalar(
                out=z_sb[:],
                in0=z_sb[:],
                scalar1=b1_t[:],
                scalar2=0.0,
                op0=mybir.AluOpType.add,
                op1=mybir.AluOpType.max,
            )

            o_ps = pp.tile([1, GE], f32, tag="o")
            nc.tensor.matmul(o_ps[:, :], lhsT=w2_t[:], rhs=z_sb[:], start=True, stop=True)
            o_sb = cp.tile([1, GE], f32, tag="os")
            nc.vector.tensor_scalar(
                out=o_sb[:],
                in0=o_ps[:, :],
                scalar1=b2_t[:],
                scalar2=None,
                op0=mybir.AluOpType.add,
            )
            nc.sync.dma_start(out=out_d[:, :], in_=o_sb[:])

    nc.compile()
    return nc


def _occurrence_ranks(key):
    """rank of each element within its equal-key group (0-based), stable."""
    order = np.argsort(key, kind="stable")
    sk = key[order]
    n = len(sk)
    if n == 0:
        return np.zeros(0, np.int64)
    starts = np.r_[0, np.flatnonzero(np.diff(sk)) + 1]
    lens = np.diff(np.r_[starts, n])
    ranks_sorted = np.arange(n) - np.repeat(starts, lens)
    ranks = np.empty(n, np.int64)
    ranks[order] = ranks_sorted
    return ranks


def prepare_inputs(x, edge_index, edge_attr, batch, W1, b1, W2, b2):
    """Host-side reformatting (placement + sparse canonicalization only)."""
    x = np.asarray(x, np.float32)
    src = np.asarray(edge_index[0], np.int64)
    dst = np.asarray(edge_index[1], np.int64)
    w = np.asarray(edge_attr, np.float32)
    batch = np.asarray(batch, np.int64)
    g = batch[dst]

    core = src // RPC
    per_core = []
    for k in range(CORES):
        m = core == k
        r = src[m] - k * RPC
        gg = g[m]
        cell_key = r * G + gg
        uniq, inv = np.unique(cell_key, return_inverse=True)
        w_cell = np.bincount(inv, weights=w[m].astype(np.float64)).astype(np.float32)
        per_core.append((uniq, w_cell))

    # count layers (core k counts nodes of its own graph range, columns global)
    node_bounds = np.searchsorted(batch, np.arange(CORES + 1) * GPC)
    ranks_all, n_layers = [], 1
    for k in range(CORES):
        n0, n1 = node_bounds[k], node_bounds[k + 1]
        gl = batch[n0:n1]
        pk = np.arange(n1 - n0) % P
        ranks = _occurrence_ranks(pk * G + gl)
        ranks_all.append((pk, ranks, gl))
        n_layers = max(n_layers, int(ranks.max(initial=-1)) + 1)

    in_maps = []
    for k in range(CORES):
        uniq, w_cell = per_core[k]
        r_c = uniq // G
        g_c = uniq % G
        # per-src-row u8 quantization; the row scale is folded into the x row
        s_row = np.zeros(RPC, np.float32)
        np.maximum.at(s_row, r_c, w_cell)
        s_row = np.where(s_row > 0, s_row, 1.0) / 255.0
        u = np.clip(np.rint(w_cell / s_row[r_c]), 0, 255).astype(np.uint8)
        cu = np.zeros((P, NW * G), np.uint8)
        cu[r_c % P, (r_c // P) * G + g_c] = u

        xk = np.zeros((NW * P, D_FEAT), np.float16)
        xk[:RPC] = (
            x[k * RPC : (k + 1) * RPC] * s_row[:, None]
        ).astype(np.float16)
        xw = np.ascontiguousarray(
            xk.reshape(NW, P, D_FEAT).transpose(1, 0, 2)
        ).reshape(P, NW * D_FEAT)

        pk, ranks, gl = ranks_all[k]
        cm = np.zeros((P, n_layers * G), np.float16)
        cm[pk, ranks * G + gl] = np.float16(1.0)

        in_maps.append(
            {
                "cu": cu,
                "xw": xw,
                "cm": cm,
                "w1": np.asarray(W1, np.float32).reshape(D_FEAT, D_HID),
                "b1": np.asarray(b1, np.float32).reshape(D_HID, 1),
                "w2": np.asarray(W2, np.float32).reshape(D_HID, 1),
                "b2": np.asarray(b2, np.float32).reshape(1, 1),
            }
        )
    return in_maps, n_layers


def get_nc(n_layers, collective=None):
    collective = collective or COLLECTIVE
    key = (n_layers, collective)
    if key not in _nc_cache:
        _nc_cache[key] = _build_nc(n_layers, collective)
    return _nc_cache[key]


def assemble(res, collective=None):
    collective = collective or COLLECTIVE
    if collective == "AR":
        out = np.asarray(res.results[0]["out"], np.float32).reshape(N_GRAPHS)
    else:
        out = np.concatenate(
            [
                np.asarray(res.results[k]["out"], np.float32).reshape(GPC)
                for k in range(CORES)
            ]
        )
    return out.reshape(N_GRAPHS, 1)


def _gout(collective=None):
    return N_GRAPHS if (collective or COLLECTIVE) == "AR" else GPC


def kernel(**inputs):
    from concourse import bass_utils

    in_maps, n_layers = prepare_inputs(**inputs)
    nc = get_nc(n_layers)
    res = bass_utils.run_bass_kernel_spmd(nc, in_maps, core_ids=list(range(CORES)))
    return assemble(res)


# revision 8
# speedup vs baseline: 2.2335x; 2.2335x over previous
"""GCNNet (SimpleConv sum-aggr + global_mean_pool + 2-layer MLP) on 8 trn2 cores.

Math: out[g] = MLP(relu(sums[g] / max(counts[g],1)))
  sums[g,:]  = sum_e w_e * x[src_e,:] * [batch[dst_e]==g]
  counts[g]  = #{i : batch[i]==g}

Sharding (v3): by graph range (64 graphs per core) -> fully independent cores,
no collective.  Host canonicalizes each core's edge list (duplicate (src,
graph) cells coalesced, one row per distinct src) and quantizes the per-row
cell weights to uint8, folding each row's scale into that row's fp16 x copy.
On device the u8 coefficient windows are cast to fp16 during the SWDGE DMA
(halving their HBM traffic vs fp16) while the x windows stream on the HWDGE
queue.  Each window is one PE matmul with the coefficient block as the
STATIONARY operand: accT[64, 96] += C_w[128,64].T @ x_w[128,96] (f32 PSUM).
Node counts per graph come from 0/1 layer matrices contracted against ones.
A PE transpose flips accT||counts to [97, 64] for the tiny-MLP epilogue.
"""

import numpy as np

N_NODES = 50000
N_EDGES = 800000
D_FEAT = 96
D_HID = 10
N_GRAPHS = 512
CORES = 8
GPC = N_GRAPHS // CORES         # 64 graphs per core
P = 128

_nc_cache = {}


def _chunks(tot_w):
    """window chunks: ramped sizes for an early PE start."""
    sizes = [4, 8, 16, 32, 48]
    out = []
    w = 0
    i = 0
    while w < tot_w:
        n = min(sizes[i] if i < len(sizes) else 64, tot_w - w)
        out.append((w, n))
        w += n
        i += 1
    return out


def _build_nc(tot_w, n_layers):
    import concourse.mybir as mybir
    import concourse.tile as tile
    from concourse import bacc

    f32 = mybir.dt.float32
    f16 = mybir.dt.float16
    u8 = mybir.dt.uint8
    G = GPC
    D = D_FEAT
    L = n_layers

    nc = bacc.Bacc(
        "TRN2",
        target_bir_lowering=False,
        debug=False,
        num_devices=CORES,
    )

    cu_d = nc.dram_tensor("cu", [P, tot_w * G], u8, kind="ExternalInput")
    xw_d = nc.dram_tensor("xw", [P, tot_w * D], f16, kind="ExternalInput")
    cm_d = nc.dram_tensor("cm", [P, L * G], f16, kind="ExternalInput")
    eye_d = nc.dram_tensor("eye", [G, G], f32, kind="ExternalInput")
    w1_d = nc.dram_tensor("w1", [D, D_HID], f32, kind="ExternalInput")
    b1_d = nc.dram_tensor("b1", [D_HID, 1], f32, kind="ExternalInput")
    w2_d = nc.dram_tensor("w2", [D_HID, 1], f32, kind="ExternalInput")
    b2_d = nc.dram_tensor("b2", [1, 1], f32, kind="ExternalInput")
    out_d = nc.dram_tensor("out", [1, G], f32, kind="ExternalOutput")

    chunks = _chunks(tot_w)

    with tile.TileContext(nc) as tc:
        with (
            tc.tile_pool(name="const", bufs=1) as cp,
            tc.tile_pool(name="xw", bufs=3) as xp,
            tc.tile_pool(name="cu8", bufs=3) as cu8p,
            tc.tile_pool(name="cw", bufs=3) as cwp,
            tc.tile_pool(name="psum", bufs=1, space="PSUM") as pp,
        ):
            accT_ps = pp.tile([G, D], f32, tag="accT")
            cntT_ps = pp.tile([G, 1], f32, tag="cntT")

            ones_t = cp.tile([P, 1], f16, tag="ones")
            nc.vector.memset(ones_t[:], 1.0)
            ones10_t = cp.tile([1, D_HID], f32, tag="ones10")
            nc.vector.memset(ones10_t[:], 1.0)

            const_c = min(2, len(chunks) - 1)
            cm_t = None
            for c, (w0, nw) in enumerate(chunks):
                cu8_t = cu8p.tile([P, 64 * G], u8, tag="cu8")
                nc.sync.dma_start(
                    out=cu8_t[:, : nw * G], in_=cu_d[:, w0 * G : (w0 + nw) * G]
                )
                ct = cwp.tile([P, 64 * G], f16, tag="cw")
                nc.vector.tensor_copy(
                    out=ct[:, : nw * G], in_=cu8_t[:, : nw * G]
                )
                xt = xp.tile([P, 64 * D], f16, tag="xw")
                nh = (nw + 1) // 2
                nc.sync.dma_start(
                    out=xt[:, : nh * D], in_=xw_d[:, w0 * D : (w0 + nh) * D]
                )
                if nw > nh:
                    nc.sync.dma_start(
                        out=xt[:, nh * D : nw * D],
                        in_=xw_d[:, (w0 + nh) * D : (w0 + nw) * D],
                    )
                if c == const_c:
                    # small consts once the pipeline is primed
                    cm_t = cp.tile([P, L * G], f16, tag="cm")
                    nc.sync.dma_start(out=cm_t[:], in_=cm_d[:, :])
                    eye_t = cp.tile([G, G], f32, tag="eye")
                    nc.sync.dma_start(out=eye_t[:], in_=eye_d[:, :])
                    w1_t = cp.tile([D, D_HID], f32, tag="w1")
                    nc.sync.dma_start(out=w1_t[:], in_=w1_d[:, :])
                    b1_t = cp.tile([D_HID, 1], f32, tag="b1")
                    nc.sync.dma_start(out=b1_t[:], in_=b1_d[:, :])
                    w2_t = cp.tile([D_HID, 1], f32, tag="w2")
                    nc.sync.dma_start(out=w2_t[:], in_=w2_d[:, :])
                    b2_t = cp.tile([1, 1], f32, tag="b2")
                    nc.sync.dma_start(out=b2_t[:], in_=b2_d[:, :])
                for lw in range(nw):
                    w = w0 + lw
                    nc.tensor.matmul(
                        accT_ps[:, :],
                        lhsT=ct[:, lw * G : (lw + 1) * G],
                        rhs=xt[:, lw * D : (lw + 1) * D],
                        start=(w == 0),
                        stop=(w == tot_w - 1),
                    )

            # node counts: cntT[g] = sum_l sum_p cm_l[p, g]
            for l in range(L):
                nc.tensor.matmul(
                    cntT_ps[:, :],
                    lhsT=cm_t[:, l * G : (l + 1) * G],
                    rhs=ones_t[:],
                    start=(l == 0),
                    stop=(l == L - 1),
                )

            # [64, 97] = [relu(accT) | cnt], then PE-transpose to [97, 64]
            a64_sb = cp.tile([G, D + 1], f32, tag="a64")
            nc.vector.tensor_scalar_max(a64_sb[:, :D], accT_ps[:, :], 0.0)
            nc.vector.tensor_copy(out=a64_sb[:, D : D + 1], in_=cntT_ps[:, :])
            fin_ps = pp.tile([D + 1, G], f32, tag="fin")
            nc.tensor.transpose(fin_ps[:, :], a64_sb[:, :], eye_t[:])
            fin_sb = cp.tile([D + 1, G], f32, tag="fins")
            nc.vector.tensor_copy(out=fin_sb[:, :], in_=fin_ps[:, :])

            # epilogue: relu already applied; positive 1/count scale commutes
            cmax = cp.tile([1, G], f32, tag="cmax")
            nc.vector.tensor_scalar_max(cmax[:], fin_sb[D : D + 1, :], 1.0)
            recip = cp.tile([1, G], f32, tag="recip")
            nc.vector.reciprocal(recip[:], cmax[:])

            b_ps = pp.tile([D_HID, G], f32, tag="b")
            nc.tensor.matmul(
                b_ps[:, :], lhsT=w1_t[:], rhs=fin_sb[:D, :], start=True, stop=True
            )
            rb_ps = pp.tile([D_HID, G], f32, tag="rb")
            nc.tensor.matmul(
                rb_ps[:, :], lhsT=ones10_t[:], rhs=recip[:], start=True, stop=True
            )
            rb_sb = cp.tile([D_HID, G], f32, tag="rbs")
            nc.vector.tensor_copy(out=rb_sb[:, :], in_=rb_ps[:, :])

            z_sb = cp.tile([D_HID, G], f32, tag="z")
            nc.vector.tensor_tensor(
                z_sb[:], b_ps[:, :], rb_sb[:], mybir.AluOpType.mult
            )
            nc.vector.tensor_scalar(
                out=z_sb[:],
                in0=z_sb[:],
                scalar1=b1_t[:],
                scalar2=0.0,
                op0=mybir.AluOpType.add,
                op1=mybir.AluOpType.max,
            )

            o_ps = pp.tile([1, G], f32, tag="o")
            nc.tensor.matmul(o_ps[:, :], lhsT=w2_t[:], rhs=z_sb[:], start=True, stop=True)
            o_sb = cp.tile([1, G], f32, tag="os")
            nc.vector.tensor_scalar(
                out=o_sb[:],
                in0=o_ps[:, :],
                scalar1=b2_t[:],
                scalar2=None,
                op0=mybir.AluOpType.add,
            )
            nc.sync.dma_start(out=out_d[:, :], in_=o_sb[:])

    nc.compile()
    return nc


def _occurrence_ranks(key):
    """rank of each element within its equal-key group (0-based), stable."""
    order = np.argsort(key, kind="stable")
    sk = key[order]
    n = len(sk)
    if n == 0:
        return np.zeros(0, np.int64)
    starts = np.r_[0, np.flatnonzero(np.diff(sk)) + 1]
    lens = np.diff(np.r_[starts, n])
    ranks_sorted = np.arange(n) - np.repeat(starts, lens)
    ranks = np.empty(n, np.int64)
    ranks[order] = ranks_sorted
    return ranks


def prepare_inputs(x, edge_index, edge_attr, batch, W1, b1, W2, b2):
    """Host-side reformatting (placement + sparse canonicalization only)."""
    G = GPC
    D = D_FEAT

    x = np.asarray(x, np.float32)
    src = np.asarray(edge_index[0], np.int64)
    dst = np.asarray(edge_index[1], np.int64)
    w = np.asarray(edge_attr, np.float32)
    batch = np.asarray(batch, np.int64)
    g = batch[dst]

    core = g // G
    per_core = []
    max_rows = 0
    # node range per core: batch is sorted
    node_bounds = np.searchsorted(batch, np.arange(CORES + 1) * G)
    n_layers = 1
    ranks_all = []
    for k in range(CORES):
        m = core == k
        sk_ = src[m]
        gk = (g[m] - k * G).astype(np.int64)
        wk = w[m].astype(np.float64)
        # coalesce duplicate (src, graph) cells; one cell per (src, g)
        cell_key = sk_ * G + gk
        uniq_cells, inv = np.unique(cell_key, return_inverse=True)
        w_cell = np.bincount(inv, weights=wk).astype(np.float32)
        src_c = uniq_cells // G
        g_c = uniq_cells % G
        # one row per distinct src
        uniq, row_of_cell = np.unique(src_c, return_inverse=True)
        max_rows = max(max_rows, len(uniq))
        per_core.append((uniq, row_of_cell, g_c, w_cell))

        n0, n1 = node_bounds[k], node_bounds[k + 1]
        bk = batch[n0:n1] - k * G
        pk = np.arange(n1 - n0) % P
        ranks = _occurrence_ranks(pk * G + bk)
        ranks_all.append((pk, ranks, bk))
        n_layers = max(n_layers, int(ranks.max(initial=-1)) + 1)

    tot_w = max(1, -(-max_rows // P))
    assert n_layers <= 8, n_layers

    in_maps = []
    for k in range(CORES):
        uniq, row_of_cell, g_c, w_cell = per_core[k]
        nrows = len(uniq)
        # per-row u8 quantization; the row scale is folded into the x row
        s_row = np.zeros(nrows, np.float32)
        np.maximum.at(s_row, row_of_cell, w_cell)
        s_row = np.where(s_row > 0, s_row, 1.0) / 255.0
        u = np.clip(np.rint(w_cell / s_row[row_of_cell]), 0, 255).astype(np.uint8)

        cu = np.zeros((P, tot_w * G), np.uint8)
        cu[row_of_cell % P, (row_of_cell // P) * G + g_c] = u

        xk = np.zeros((tot_w * P, D), np.float16)
        xk[:nrows] = (x[uniq] * s_row[:, None]).astype(np.float16)
        xw = np.ascontiguousarray(
            xk.reshape(tot_w, P, D).transpose(1, 0, 2)
        ).reshape(P, tot_w * D)

        pk, ranks, bk = ranks_all[k]
        cm = np.zeros((P, n_layers * G), np.float16)
        cm[pk, ranks * G + bk] = 1.0

        in_maps.append(
            {
                "cu": cu,
                "xw": xw,
                "cm": cm,
                "eye": np.eye(G, dtype=np.float32),
                "w1": np.asarray(W1, np.float32).reshape(D, D_HID),
                "b1": np.asarray(b1, np.float32).reshape(D_HID, 1),
                "w2": np.asarray(W2, np.float32).reshape(D_HID, 1),
                "b2": np.asarray(b2, np.float32).reshape(1, 1),
            }
        )
    return in_maps, tot_w, n_layers


def get_nc(tot_w, n_layers):
    key = (tot_w, n_layers)
    if key not in _nc_cache:
        _nc_cache[key] = _build_nc(tot_w, n_layers)
    return _nc_cache[key]


def assemble(res):
    out = np.concatenate(
        [
            np.asarray(res.results[k]["out"], np.float32).reshape(GPC)
            for k in range(CORES)
        ]
    )
    return out.reshape(N_GRAPHS, 1)


def kernel(**inputs):
    from concourse import bass_utils

    in_maps, tot_w, n_layers = prepare_inputs(**inputs)
    nc = get_nc(tot_w, n_layers)
    res = bass_utils.run_bass_kernel_spmd(nc, in_maps, core_ids=list(range(CORES)))
    return assemble(res)


# revision 14
# speedup vs baseline: 2.7078x; 1.2123x over previous
"""GCNNet (SimpleConv sum-aggr + global_mean_pool + 2-layer MLP) on 8 trn2 cores.

Math: out[g] = MLP(relu(sums[g] / max(counts[g],1)))
  sums[g,:]  = sum_e w_e * x[src_e,:] * [batch[dst_e]==g]
  counts[g]  = #{i : batch[i]==g}

Sharding (v3): by graph range (64 graphs per core) -> fully independent cores,
no collective.  Host canonicalizes each core's edge list (duplicate (src,
graph) cells coalesced, one row per distinct src) and quantizes the per-row
cell weights to uint8, folding each row's scale into that row's fp16 x copy.
On device the u8 coefficient windows are cast to fp16 during the SWDGE DMA
(halving their HBM traffic vs fp16) while the x windows stream on the HWDGE
queue.  Each window is one PE matmul with the coefficient block as the
STATIONARY operand: accT[64, 96] += C_w[128,64].T @ x_w[128,96] (f32 PSUM).
Node counts per graph come from 0/1 layer matrices contracted against ones.
A PE transpose flips accT||counts to [97, 64] for the tiny-MLP epilogue.
"""

import numpy as np

N_NODES = 50000
N_EDGES = 800000
D_FEAT = 96
D_HID = 10
N_GRAPHS = 512
CORES = 8
GPC = N_GRAPHS // CORES         # 64 graphs per core
P = 128

_nc_cache = {}


def _chunks(tot_w):
    """window chunks: ramped sizes for an early PE start."""
    sizes = [8, 16, 32]
    out = []
    w = 0
    i = 0
    while w < tot_w:
        n = min(sizes[i] if i < len(sizes) else 64, tot_w - w)
        out.append((w, n))
        w += n
        i += 1
    return out


def _build_nc(tot_w, n_layers):
    import concourse.mybir as mybir
    import concourse.tile as tile
    from concourse import bacc

    f32 = mybir.dt.float32
    f16 = mybir.dt.float16
    u8 = mybir.dt.uint8
    G = GPC
    D = D_FEAT
    L = n_layers

    nc = bacc.Bacc(
        "TRN2",
        target_bir_lowering=False,
        debug=False,
        num_devices=CORES,
    )

    # fused per-window layout: [x row bytes (96 fp16 = 192B) | u8 cells (64B)]
    WB = 2 * D + G
    xc_d = nc.dram_tensor("xc", [P, tot_w * WB], u8, kind="ExternalInput")
    cm_d = nc.dram_tensor("cm", [P, L * G], f16, kind="ExternalInput")
    eye_d = nc.dram_tensor("eye", [G, G], f32, kind="ExternalInput")
    w1_d = nc.dram_tensor("w1", [D, D_HID], f32, kind="ExternalInput")
    b1_d = nc.dram_tensor("b1", [D_HID, 1], f32, kind="ExternalInput")
    w2_d = nc.dram_tensor("w2", [D_HID, 1], f32, kind="ExternalInput")
    b2_d = nc.dram_tensor("b2", [1, 1], f32, kind="ExternalInput")
    out_d = nc.dram_tensor("out", [1, G], f32, kind="ExternalOutput")

    chunks = _chunks(tot_w)

    with tile.TileContext(nc) as tc:
        with (
            tc.tile_pool(name="const", bufs=1) as cp,
            tc.tile_pool(name="cu8", bufs=3) as cu8p,
            tc.tile_pool(name="cw", bufs=3) as cwp,
            tc.tile_pool(name="psum", bufs=1, space="PSUM") as pp,
        ):
            accT_ps = pp.tile([G, D], f32, tag="accT")
            cntT_ps = pp.tile([G, 1], f32, tag="cntT")

            ones_t = cp.tile([P, 1], f16, tag="ones")
            nc.vector.memset(ones_t[:], 1.0)
            ones10_t = cp.tile([1, D_HID], f32, tag="ones10")
            nc.vector.memset(ones10_t[:], 1.0)

            const_c = min(2, len(chunks) - 1)
            cm_t = None
            for c, (w0, nw) in enumerate(chunks):
                xc_t = cu8p.tile([P, 64 * WB], u8, tag="xc")
                nc.sync.dma_start(
                    out=xc_t[:, : nw * WB], in_=xc_d[:, w0 * WB : (w0 + nw) * WB]
                )
                ct = cwp.tile([P, 64 * G], f16, tag="cw")
                nc.vector.tensor_copy(
                    out=ct[:, : nw * G].rearrange("p (w b) -> p w b", b=G),
                    in_=xc_t[:, : nw * WB].rearrange("p (w b) -> p w b", b=WB)[
                        :, :, 2 * D : WB
                    ],
                )
                if c == const_c:
                    # small consts once the pipeline is primed
                    cm_t = cp.tile([P, L * G], f16, tag="cm")
                    nc.sync.dma_start(out=cm_t[:], in_=cm_d[:, :])
                    eye_t = cp.tile([G, G], f32, tag="eye")
                    nc.sync.dma_start(out=eye_t[:], in_=eye_d[:, :])
                    w1_t = cp.tile([D, D_HID], f32, tag="w1")
                    nc.sync.dma_start(out=w1_t[:], in_=w1_d[:, :])
                    b1_t = cp.tile([D_HID, 1], f32, tag="b1")
                    nc.sync.dma_start(out=b1_t[:], in_=b1_d[:, :])
                    w2_t = cp.tile([D_HID, 1], f32, tag="w2")
                    nc.sync.dma_start(out=w2_t[:], in_=w2_d[:, :])
                    b2_t = cp.tile([1, 1], f32, tag="b2")
                    nc.sync.dma_start(out=b2_t[:], in_=b2_d[:, :])
                for lw in range(nw):
                    w = w0 + lw
                    nc.tensor.matmul(
                        accT_ps[:, :],
                        lhsT=ct[:, lw * G : (lw + 1) * G],
                        rhs=xc_t[:, lw * WB : lw * WB + 2 * D].bitcast(f16),
                        start=(w == 0),
                        stop=(w == tot_w - 1),
                    )

            # node counts: cntT[g] = sum_l sum_p cm_l[p, g]
            for l in range(L):
                nc.tensor.matmul(
                    cntT_ps[:, :],
                    lhsT=cm_t[:, l * G : (l + 1) * G],
                    rhs=ones_t[:],
                    start=(l == 0),
                    stop=(l == L - 1),
                )

            # [64, 97] = [relu(accT) | cnt], then PE-transpose to [97, 64]
            a64_sb = cp.tile([G, D + 1], f32, tag="a64")
            nc.vector.tensor_scalar_max(a64_sb[:, :D], accT_ps[:, :], 0.0)
            nc.vector.tensor_copy(out=a64_sb[:, D : D + 1], in_=cntT_ps[:, :])
            fin_ps = pp.tile([D + 1, G], f32, tag="fin")
            nc.tensor.transpose(fin_ps[:, :], a64_sb[:, :], eye_t[:])
            fin_sb = cp.tile([D + 1, G], f32, tag="fins")
            nc.vector.tensor_copy(out=fin_sb[:, :], in_=fin_ps[:, :])

            # epilogue: relu already applied; positive 1/count scale commutes
            cmax = cp.tile([1, G], f32, tag="cmax")
            nc.vector.tensor_scalar_max(cmax[:], fin_sb[D : D + 1, :], 1.0)
            recip = cp.tile([1, G], f32, tag="recip")
            nc.vector.reciprocal(recip[:], cmax[:])

            b_ps = pp.tile([D_HID, G], f32, tag="b")
            nc.tensor.matmul(
                b_ps[:, :], lhsT=w1_t[:], rhs=fin_sb[:D, :], start=True, stop=True
            )
            rb_ps = pp.tile([D_HID, G], f32, tag="rb")
            nc.tensor.matmul(
                rb_ps[:, :], lhsT=ones10_t[:], rhs=recip[:], start=True, stop=True
            )
            rb_sb = cp.tile([D_HID, G], f32, tag="rbs")
            nc.vector.tensor_copy(out=rb_sb[:, :], in_=rb_ps[:, :])

            z_sb = cp.tile([D_HID, G], f32, tag="z")
            nc.vector.tensor_tensor(
                z_sb[:], b_ps[:, :], rb_sb[:], mybir.AluOpType.mult
            )
            nc.vector.tensor_scalar(
                out=z_sb[:],
                in0=z_sb[:],
                scalar1=b1_t[:],
                scalar2=0.0,
                op0=mybir.AluOpType.add,
                op1=mybir.AluOpType.max,
            )

            o_ps = pp.tile([1, G], f32, tag="o")
            nc.tensor.matmul(o_ps[:, :], lhsT=w2_t[:], rhs=z_sb[:], start=True, stop=True)
            o_sb = cp.tile([1, G], f32, tag="os")
            nc.vector.tensor_scalar(
                out=o_sb[:],
                in0=o_ps[:, :],
                scalar1=b2_t[:],
                scalar2=None,
                op0=mybir.AluOpType.add,
            )
            nc.sync.dma_start(out=out_d[:, :], in_=o_sb[:])

    nc.compile()
    return nc


def _occurrence_ranks(key):
    """rank of each element within its equal-key group (0-based), stable."""
    order = np.argsort(key, kind="stable")
    sk = key[order]
    n = len(sk)
    if n == 0:
        return np.zeros(0, np.int64)
    starts = np.r_[0, np.flatnonzero(np.diff(sk)) + 1]
    lens = np.diff(np.r_[starts, n])
    ranks_sorted = np.arange(n) - np.repeat(starts, lens)
    ranks = np.empty(n, np.int64)
    ranks[order] = ranks_sorted
    return ranks


def prepare_inputs(x, edge_index, edge_attr, batch, W1, b1, W2, b2):
    """Host-side reformatting (placement + sparse canonicalization only)."""
    G = GPC
    D = D_FEAT

    x = np.asarray(x, np.float32)
    src = np.asarray(edge_index[0], np.int64)
    dst = np.asarray(edge_index[1], np.int64)
    w = np.asarray(edge_attr, np.float32)
    batch = np.asarray(batch, np.int64)
    g = batch[dst]

    core = g // G
    per_core = []
    max_rows = 0
    # node range per core: batch is sorted
    node_bounds = np.searchsorted(batch, np.arange(CORES + 1) * G)
    n_layers = 1
    ranks_all = []
    for k in range(CORES):
        m = core == k
        sk_ = src[m]
        gk = (g[m] - k * G).astype(np.int64)
        wk = w[m].astype(np.float64)
        # coalesce duplicate (src, graph) cells; one cell per (src, g)
        cell_key = sk_ * G + gk
        uniq_cells, inv = np.unique(cell_key, return_inverse=True)
        w_cell = np.bincount(inv, weights=wk).astype(np.float32)
        src_c = uniq_cells // G
        g_c = uniq_cells % G
        # one row per distinct src
        uniq, row_of_cell = np.unique(src_c, return_inverse=True)
        max_rows = max(max_rows, len(uniq))
        per_core.append((uniq, row_of_cell, g_c, w_cell))

        n0, n1 = node_bounds[k], node_bounds[k + 1]
        bk = batch[n0:n1] - k * G
        pk = np.arange(n1 - n0) % P
        ranks = _occurrence_ranks(pk * G + bk)
        ranks_all.append((pk, ranks, bk))
        n_layers = max(n_layers, int(ranks.max(initial=-1)) + 1)

    tot_w = max(1, -(-max_rows // P))
    assert n_layers <= 8, n_layers

    in_maps = []
    for k in range(CORES):
        uniq, row_of_cell, g_c, w_cell = per_core[k]
        nrows = len(uniq)
        # per-row u8 quantization; the row scale is folded into the x row
        s_row = np.zeros(nrows, np.float32)
        np.maximum.at(s_row, row_of_cell, w_cell)
        s_row = np.where(s_row > 0, s_row, 1.0) / 255.0
        u = np.clip(np.rint(w_cell / s_row[row_of_cell]), 0, 255).astype(np.uint8)

        cu = np.zeros((P, tot_w, G), np.uint8)
        cu[row_of_cell % P, row_of_cell // P, g_c] = u

        xk = np.zeros((tot_w * P, D), np.float16)
        xk[:nrows] = (x[uniq] * s_row[:, None]).astype(np.float16)
        xw = np.ascontiguousarray(
            xk.reshape(tot_w, P, D).transpose(1, 0, 2)
        )  # [P, tot_w, D] fp16

        # fused per-window layout: [x bytes (192) | u8 cells (64)]
        WB = 2 * D + G
        xc = np.empty((P, tot_w, WB), np.uint8)
        xc[:, :, : 2 * D] = xw.view(np.uint8).reshape(P, tot_w, 2 * D)
        xc[:, :, 2 * D :] = cu
        xc = xc.reshape(P, tot_w * WB)

        pk, ranks, bk = ranks_all[k]
        cm = np.zeros((P, n_layers * G), np.float16)
        cm[pk, ranks * G + bk] = 1.0

        in_maps.append(
            {
                "xc": xc,
                "cm": cm,
                "eye": np.eye(G, dtype=np.float32),
                "w1": np.asarray(W1, np.float32).reshape(D, D_HID),
                "b1": np.asarray(b1, np.float32).reshape(D_HID, 1),
                "w2": np.asarray(W2, np.float32).reshape(D_HID, 1),
                "b2": np.asarray(b2, np.float32).reshape(1, 1),
            }
        )
    return in_maps, tot_w, n_layers


def get_nc(tot_w, n_layers):
    key = (tot_w, n_layers)
    if key not in _nc_cache:
        _nc_cache[key] = _build_nc(tot_w, n_layers)
    return _nc_cache[key]


def assemble(res):
    out = np.concatenate(
        [
            np.asarray(res.results[k]["out"], np.float32).reshape(GPC)
            for k in range(CORES)
        ]
    )
    return out.reshape(N_GRAPHS, 1)


def kernel(**inputs):
    from concourse import bass_utils

    in_maps, tot_w, n_layers = prepare_inputs(**inputs)
    nc = get_nc(tot_w, n_layers)
    res = bass_utils.run_bass_kernel_spmd(nc, in_maps, core_ids=list(range(CORES)))
    return assemble(res)
